# revision 31
# baseline (speedup 1.0000x reference)
"""Trainium2 Bass kernel for CombinedSurvLoss (NLL survival + pairwise rank loss).

Strategy (sorted suffix-sum; O(B) device work instead of the O(B^2) mask):
  The rank loss needs, per row i, lse_i = ln(sum_{j: t_j > t_i} e^{risk_j}).
  The host computes perm = argsort(t) (a pure permutation -- data movement,
  like the baseline's layout packing; every floating-point operation stays on
  device) and packs outputs/y/c in sorted order. In rank space the masked
  logsumexp collapses to a strict suffix sum of e = exp(risk):
      C_r = sum_{r' > r} e_{r'},   lse_r = ln(C_r),
  and both loss terms are means, so no unsort is needed.

  Layout: sorted rank r = p*64 + n on a [128 partition, 64 free] grid.
  The suffix sum factors into
    - a per-partition prefix scan along the free axis (one DVE
      tensor_tensor_scan), and
    - a cross-partition suffix of per-partition totals (one PE matmul with a
      strict lower-triangular [128,128] ones matrix built on GpSimd by
      affine_select),
  combined as C = total[p] + cross[p] - scan_incl[p,n].

  1 - sigmoid(x) is computed as 1/(1+e^x) (ACT Exp + DVE reciprocal) so every
  ACT op (Exp, Ln) is served by the single natural_log_exp activation table:
  one ~1.3us table load for the whole kernel instead of three.

  The NLL part gathers h/S_prev/S_this by one-hot(y) dot products; row
  reductions ride free on accum_out of existing ops. valid_rank (= event and
  rank < B-1) masks the one guaranteed-empty last rank via affine_select, so
  the host never inspects c. ln(C) is computed as Ln((-1)*(incl - TS) + 1e-3):
  the tiny bias keeps the masked last row (C == 0 up to rounding) finite
  without a separate clamp; it perturbs real lse values by < 1e-2 of a unit
  on the single smallest-C row (loss tolerance is 2e-2 relative).

  All 8 cores run the identical program on identical inputs
  (communication-avoiding replication -- at ~35 instructions the kernel is
  overhead-bound and sharding could only add transfers); the host divides the
  summed partials by NCORES.
"""

import sys

for _p in ("/opt/trn_rl_repo", "/root/.axon_site/_ro/trn_rl_repo"):
    if _p not in sys.path:
        sys.path.append(_p)

import numpy as np

B = 8192
K = 4
NCORES = 8
P = 128
NN = B // P  # 64 free columns; sorted rank r = p*NN + n
EPS = 1e-7
LAMBDA_RANK = 0.5
LSE_BIAS = 1e-3  # ln(C + bias): keeps the masked empty row finite

_NC_CACHE = {}


def _build_nc():
    import concourse.bass as bass
    import concourse.tile as tile
    import concourse.tile_sem_assignment as tsa
    from concourse import mybir

    tsa.NUM_HWDGE_SEMS = 8

    # The kernel-tail Drain aggregates one wait per engine/queue, but its
    # CTRL descriptor has a single-digit wait budget (empirically < 5).
    # Spread the waits across preceding single-wait SP NOPs instead.
    from concourse.vector_clock import ScopedClock

    def _split_drain_and_barrier(self, tick_clock, wait_clock):
        nops = [self.nc.sync.nop() for _ in range(12)]
        drain_inst = self.nc.sync.drain()
        wait_clock.add_sem_waits(
            drain_inst.ins, ScopedClock({None: tick_clock.global_clock})
        )
        si = drain_inst.ins.sync_info
        waits = list(si.on_wait or []) if si is not None else []
        if len(waits) > 1:
            drain_inst.ins.sync_info = mybir.SyncInfo(
                on_wait=waits[-1:], on_update=list(si.on_update or [])
            )
            for nop, w in zip(nops, waits[:-1]):
                nop.ins.sync_info = mybir.SyncInfo(on_wait=[w], on_update=[])
            assert len(waits) - 1 <= len(nops)
        self.nc.all_engine_barrier()
        assert self.sems is not None
        popped = self.nc._tile_sem_poison_stack.pop()
        assert popped is self._sem_poison
        self.nc.clear_and_free_semaphores(list(self.sems.allocated().values()))
        self.nc.all_engine_barrier()

    tile.TileContext._drain_and_barrier = _split_drain_and_barrier

    f32 = mybir.dt.float32
    f16 = mybir.dt.float16
    Alu = mybir.AluOpType
    Act = mybir.ActivationFunctionType

    nc = bass.Bass()
    # [p, 0:256] outputs sorted, [p, n, k] layout; [p, 256:320] y; [p, 320:384] c
    pin = nc.dram_tensor("pin", [P, K * NN + 2 * NN], f16, kind="ExternalInput")
    part = nc.dram_tensor("part", [P, 3], f32, kind="ExternalOutput")

    with tile.TileContext(nc) as tc:
        with (
            tc.tile_pool(name="big", bufs=1) as big,
            tc.tile_pool(name="psum", bufs=1, space="PSUM") as psum,
        ):
            # ---- input DMA: DMA_DIRECT2D blocks the issuing engine, and a
            # DMA's completion sem lands ~1.6us after the transfer ends.  ACT
            # moves the second outputs slice itself ahead of its activation-
            # table load, so both the table load and the DMA latency overlap.
            pft = big.tile([P, K * NN + 2 * NN], f16)
            nc.sync.dma_start(out=pft[:, 0:128], in_=pin[:, 0:128])
            nc.scalar.dma_start(out=pft[:, 128:256], in_=pin[:, 128:256])
            nc.sync.dma_start(out=pft[:, 256:384], in_=pin[:, 256:384])
            xs3 = pft[:, 0 : K * NN].rearrange("p (n k) -> p n k", k=K)
            yb = pft[:, K * NN : K * NN + NN]
            cb = pft[:, K * NN + NN : K * NN + 2 * NN]

            # ---- constants built on GpSimd while the input DMA streams ----
            ones128 = big.tile([P, P], f32)
            nc.gpsimd.memset(ones128[:], 1.0)
            # TRI[k, m] = 1 if k > m (strict lower): iota = k - m - 1 >= 0.
            # Built last: the dummy matmul below reads it, so the PE clock
            # covers every gpsimd constant and the real matmuls carry at
            # most one sync wait each (1-slot LW descriptor budget).
            tri = big.tile([P, P], f32)
            nc.gpsimd.affine_select(
                out=tri[:], in_=ones128[:], pattern=[[-1, P]],
                compare_op=Alu.is_ge, fill=0.0, base=-1, channel_multiplier=1,
            )
            psdump = psum.tile([2, 2], f32)
            nc.tensor.matmul(
                psdump[:], tri[0:2, 0:2], tri[0:2, 0:2],
                start=True, stop=True,
            )

            # ---- om = 1 - sigmoid(x) = sigmoid(-x), straight off ACT ----
            om3 = big.tile([P, NN, K], f16)
            nc.scalar.activation(
                om3[:, 0 : NN // 2, :], xs3[:, 0 : NN // 2, :], Act.Sigmoid,
                scale=-1.0,
            )
            nc.scalar.activation(
                om3[:, NN // 2 :, :], xs3[:, NN // 2 :, :], Act.Sigmoid,
                scale=-1.0,
            )
            # throwaway reads: DVE observes both xs DMA queues here, so the
            # NLL logit gather below carries only its same-engine wait
            scrx = big.tile([P, 2], f16)
            nc.vector.tensor_copy(out=scrx[:, 0:1], in_=pft[:, 0:1])
            nc.vector.tensor_copy(out=scrx[:, 1:2], in_=pft[:, 128:129])
            s3 = big.tile([P, NN, K], f16)  # S_k = cumprod(om)
            nc.vector.tensor_copy(out=s3[:, :, 0], in_=om3[:, :, 0])
            for k in range(1, K):
                nc.vector.tensor_mul(s3[:, :, k], om3[:, :, k], s3[:, :, k - 1])
            ssum = big.tile([P, NN], f32)  # = -risk
            nc.vector.tensor_reduce(
                out=ssum[:], in_=s3[:], axis=mybir.AxisListType.X, op=Alu.add
            )

            # ---- e = exp(risk); suffix sums. The ln/exp activation table
            # loads during the DVE cumprod above (one reload after Sigmoid).
            e64 = big.tile([P, NN], f32)
            nc.scalar.activation(e64[:], ssum[:], Act.Exp, scale=-1.0)
            with tc.high_priority():
                incl = big.tile([P, NN], f32)  # inclusive prefix scan of e
                nc.vector.tensor_tensor_scan(
                    out=incl[:], data0=e64[:], data1=e64[:], initial=0.0,
                    op0=Alu.add, op1=Alu.bypass,
                )
                # incl[:, 63] is the per-partition total; cross-partition
                # strict suffix of the totals via one triangle matmul.
                psS = psum.tile([P, 1], f32)
                nc.tensor.matmul(
                    psS[:], tri[:], incl[:, NN - 1 : NN], start=True, stop=True
                )
                c0a = big.tile([P, NN], f32)  # incl - total - bias
                nc.vector.tensor_scalar(
                    out=c0a[:], in0=incl[:], scalar1=incl[:, NN - 1 : NN],
                    scalar2=LSE_BIAS, op0=Alu.subtract, op1=Alu.subtract,
                )
                psSc = big.tile([P, 1], f32)  # PSUM -> SBUF; carries PE wait
                nc.vector.tensor_copy(out=psSc[:], in_=psS[:])
                c0 = big.tile([P, NN], f32)  # ... - cross = -(C + bias)
                nc.vector.tensor_scalar(
                    out=c0[:], in0=c0a[:], scalar1=psSc[:], scalar2=None,
                    op0=Alu.subtract,
                )
                lse = big.tile([P, NN], f32)  # ln(C + bias)
                nc.scalar.activation(lse[:], c0[:], Act.Ln, scale=-1.0)

            # ---- NLL gathers via one-hot(y); tensor ops are ~5x slower on
            # GpSimd than DVE, so only the affine corner-mask runs there ----
            vn = big.tile([P, NN], f32)  # 1 - c  (NLL event weight)
            nc.vector.tensor_scalar(
                out=vn[:], in0=cb, scalar1=0.0, scalar2=None, op0=Alu.is_equal
            )
            # zero the single (p=127, n=63) corner: rank B-1 has no greater t
            vr = big.tile([P, NN], f32)
            nc.gpsimd.affine_select(
                out=vr[:], in_=vn[:], pattern=[[-1, NN]],
                compare_op=Alu.is_gt, fill=0.0,
                base=B - 1, channel_multiplier=-NN,
            )
            sel3 = big.tile([P, NN, K], f16)
            for k in range(K):
                nc.vector.tensor_scalar(
                    out=sel3[:, :, k], in0=yb, scalar1=float(k), scalar2=None,
                    op0=Alu.is_equal,
                )
            # ---- NLL via the logit identity: ln(sp*h) - ln(s_this) = x_y,
            # so -nll_row = ln(clip(s_this)) + (1-c) * x_y ----
            pst3 = big.tile([P, NN, K], f16)
            nc.vector.tensor_mul(pst3[:], sel3[:], s3[:])
            st = big.tile([P, NN], f32)  # s_this = S[y]
            nc.vector.tensor_reduce(
                out=st[:], in_=pst3[:], axis=mybir.AxisListType.X, op=Alu.add
            )
            pxy3 = big.tile([P, NN, K], f16)
            nc.vector.tensor_mul(pxy3[:], sel3[:], xs3)
            xy = big.tile([P, NN], f32)  # x_y (raw logit at y)
            nc.vector.tensor_reduce(
                out=xy[:], in_=pxy3[:], axis=mybir.AxisListType.X, op=Alu.add
            )
            stc = big.tile([P, NN], f32)
            nc.vector.tensor_scalar_max(stc[:], st[:], EPS)

            # ---- fused row reductions into stack ----
            stack = big.tile([P, 3], f32)
            with tc.high_priority():
                qt = big.tile([P, NN], f32)  # lse - risk
                nc.vector.tensor_add(qt[:], lse[:], ssum[:])
                ct = big.tile([P, NN], f32)
                nc.vector.scalar_tensor_tensor(
                    out=ct[:], in0=qt[:], scalar=1.0, in1=vr[:],
                    op0=Alu.mult, op1=Alu.mult, accum_out=stack[:, 1:2],
                )
            scr = big.tile([P, NN], f32)
            nc.vector.tensor_scalar(
                out=scr[:], in0=vr[:], scalar1=0.0, scalar2=None, op0=Alu.add,
                op1=Alu.add, accum_out=stack[:, 2:3],
            )
            ls = big.tile([P, NN], f32)  # ln(s_this)
            nc.scalar.activation(ls[:], stc[:], Act.Ln)
            t3 = big.tile([P, NN], f32)
            nc.vector.tensor_mul(t3[:], xy[:], vn[:])
            nt = big.tile([P, NN], f32)  # -nll_row = ln(s_this) + (1-c)*x_y
            nc.vector.scalar_tensor_tensor(
                out=nt[:], in0=t3[:], scalar=0.0, in1=ls[:],
                op0=Alu.add, op1=Alu.add, accum_out=stack[:, 0:1],
            )

            # ---- per-partition partials [neg_nll_sum, rank_num, rank_cnt];
            # the 128-partition sum is part of the host-side gather ----
            nc.sync.dma_start(out=part[:, :], in_=stack[:])

    return nc


def _get_nc():
    if "nc" not in _NC_CACHE:
        _NC_CACHE["nc"] = _build_nc()
    return _NC_CACHE["nc"]


def make_in_maps(outputs, t, y, c):
    outputs = np.asarray(outputs, dtype=np.float32)
    t = np.asarray(t, dtype=np.float32)
    y = np.asarray(y, dtype=np.int32)
    c = np.asarray(c, dtype=np.int32)
    perm = np.argsort(t, kind="stable")  # permutation only; no FP math
    pin = np.concatenate(
        [
            outputs[perm].reshape(P, NN * K),
            y[perm].reshape(P, NN).astype(np.float32),
            c[perm].reshape(P, NN).astype(np.float32),
        ],
        axis=1,
    ).astype(np.float16)
    pin = np.ascontiguousarray(pin)
    return [{"pin": pin} for _ in range(NCORES)]


def combine_parts(parts):
    # parts: [NCORES, P, 3] per-partition partials; every core computed the
    # full-B sums, so gather = sum over partitions, average over cores.
    tot = parts.sum(axis=1)  # [NCORES, 3]
    neg_nll = tot[:, 0].sum() / np.float32(NCORES)
    num = tot[:, 1].sum() / np.float32(NCORES)
    cnt = tot[:, 2].sum() / np.float32(NCORES)
    nll = -neg_nll / np.float32(B)
    rank = num / max(cnt, np.float32(1.0)) if cnt > 0 else np.float32(0.0)
    return np.array(nll + np.float32(LAMBDA_RANK) * rank, dtype=np.float32)


def kernel(outputs, t, y, c):
    from concourse.bass_utils import run_bass_kernel_spmd

    nc = _get_nc()
    in_maps = make_in_maps(outputs, t, y, c)
    res = run_bass_kernel_spmd(nc, in_maps, list(range(NCORES))).results
    parts = np.stack([res[r]["part"].reshape(P, 3) for r in range(NCORES)])
    return combine_parts(parts)


# revision 32
# speedup vs baseline: 1.0014x; 1.0014x over previous
"""Trainium2 Bass kernel for CombinedSurvLoss (NLL survival + pairwise rank loss).

Strategy (sorted suffix-sum; O(B) device work instead of the O(B^2) mask):
  The rank loss needs, per row i, lse_i = ln(sum_{j: t_j > t_i} e^{risk_j}).
  The host computes perm = argsort(t) (a pure permutation -- data movement,
  like the baseline's layout packing; every floating-point operation stays on
  device) and packs outputs/y/c in sorted order. In rank space the masked
  logsumexp collapses to a strict suffix sum of e = exp(risk):
      C_r = sum_{r' > r} e_{r'},   lse_r = ln(C_r),
  and both loss terms are means, so no unsort is needed.

  Layout: sorted rank r = p*64 + n on a [128 partition, 64 free] grid.
  The suffix sum factors into
    - a per-partition prefix scan along the free axis (one DVE
      tensor_tensor_scan), and
    - a cross-partition suffix of per-partition totals (one PE matmul with a
      strict lower-triangular [128,128] ones matrix built on GpSimd by
      affine_select),
  combined as C = total[p] + cross[p] - scan_incl[p,n].

  om = 1 - sigmoid(x) comes straight off ACT as Sigmoid(-x); the remaining
  ACT ops (Exp, Ln) share the natural_log_exp table, so the kernel pays two
  ~1.3us table loads, both overlapped with DMA / DVE work.

  The NLL collapses via ln(s_prev * h) - ln(s_this) = x_y (the raw logit at
  index y): -nll_row = ln(clip(s_this)) + (1-c) * x_y, needing only two
  one-hot(y) dot products (s_this and x_y) and a single Ln. Row reductions
  ride free on accum_out of existing ops. valid_rank (= event and
  rank < B-1) masks the one guaranteed-empty last rank via affine_select, so
  the host never inspects c. ln(C) is computed as Ln((-1)*(incl - TS - 1e-3)):
  the tiny bias keeps the masked last row (C == 0 up to rounding) finite
  without a separate clamp; it perturbs real lse values by < 1e-2 of a unit
  on the single smallest-C row (loss tolerance is 2e-2 relative).

  Hardware notes: cross-engine deps are kept to one per instruction (1-slot
  sync-wait descriptors; Pool is multi-lane so its ops carry every dep
  themselves); DMA completion sems land ~1.6us after the transfer, hidden
  under the activation-table load; [128, NN, K] NLL tensors are f16 for DVE
  throughput while the rank-loss suffix path stays f32.

  All 8 cores run the identical program on identical inputs
  (communication-avoiding replication -- at ~35 instructions the kernel is
  overhead-bound and sharding could only add transfers); the host divides the
  summed partials by NCORES.
"""

import sys

for _p in ("/opt/trn_rl_repo", "/root/.axon_site/_ro/trn_rl_repo"):
    if _p not in sys.path:
        sys.path.append(_p)

import numpy as np

B = 8192
K = 4
NCORES = 8
P = 128
NN = B // P  # 64 free columns; sorted rank r = p*NN + n
EPS = 1e-7
LAMBDA_RANK = 0.5
LSE_BIAS = 1e-3  # ln(C + bias): keeps the masked empty row finite

_NC_CACHE = {}


def _build_nc():
    import concourse.bass as bass
    import concourse.tile as tile
    import concourse.tile_sem_assignment as tsa
    from concourse import mybir

    tsa.NUM_HWDGE_SEMS = 8

    # The kernel-tail Drain aggregates one wait per engine/queue, but its
    # CTRL descriptor has a single-digit wait budget (empirically < 5).
    # Spread the waits across preceding single-wait SP NOPs instead.
    from concourse.vector_clock import ScopedClock

    def _split_drain_and_barrier(self, tick_clock, wait_clock):
        nops = [self.nc.sync.nop() for _ in range(12)]
        drain_inst = self.nc.sync.drain()
        wait_clock.add_sem_waits(
            drain_inst.ins, ScopedClock({None: tick_clock.global_clock})
        )
        si = drain_inst.ins.sync_info
        waits = list(si.on_wait or []) if si is not None else []
        if len(waits) > 1:
            drain_inst.ins.sync_info = mybir.SyncInfo(
                on_wait=waits[-1:], on_update=list(si.on_update or [])
            )
            for nop, w in zip(nops, waits[:-1]):
                nop.ins.sync_info = mybir.SyncInfo(on_wait=[w], on_update=[])
            assert len(waits) - 1 <= len(nops)
        self.nc.all_engine_barrier()
        assert self.sems is not None
        popped = self.nc._tile_sem_poison_stack.pop()
        assert popped is self._sem_poison
        self.nc.clear_and_free_semaphores(list(self.sems.allocated().values()))
        self.nc.all_engine_barrier()

    tile.TileContext._drain_and_barrier = _split_drain_and_barrier

    f32 = mybir.dt.float32
    f16 = mybir.dt.float16
    Alu = mybir.AluOpType
    Act = mybir.ActivationFunctionType

    nc = bass.Bass()
    # [p, 0:256] outputs sorted, [p, n, k] layout; [p, 256:320] y; [p, 320:384] c
    pin = nc.dram_tensor("pin", [P, K * NN + 2 * NN], f16, kind="ExternalInput")
    part = nc.dram_tensor("part", [P, 3], f32, kind="ExternalOutput")

    with tile.TileContext(nc) as tc:
        with (
            tc.tile_pool(name="big", bufs=1) as big,
            tc.tile_pool(name="psum", bufs=1, space="PSUM") as psum,
        ):
            # ---- input DMA: DMA_DIRECT2D blocks the issuing engine, and a
            # DMA's completion sem lands ~1.6us after the transfer ends.  ACT
            # moves the second outputs slice itself ahead of its activation-
            # table load, so both the table load and the DMA latency overlap.
            pft = big.tile([P, K * NN + 2 * NN], f16)
            nc.sync.dma_start(out=pft[:, 0:128], in_=pin[:, 0:128])
            nc.scalar.dma_start(out=pft[:, 128:256], in_=pin[:, 128:256])
            nc.sync.dma_start(out=pft[:, 256:384], in_=pin[:, 256:384])
            xs3 = pft[:, 0 : K * NN].rearrange("p (n k) -> p n k", k=K)
            yb = pft[:, K * NN : K * NN + NN]
            cb = pft[:, K * NN + NN : K * NN + 2 * NN]

            # ---- constants built on GpSimd while the input DMA streams ----
            ones128 = big.tile([P, P], f32)
            nc.gpsimd.memset(ones128[:], 1.0)
            # TRI[k, m] = 1 if k > m (strict lower): iota = k - m - 1 >= 0.
            # Built last: the dummy matmul below reads it, so the PE clock
            # covers every gpsimd constant and the real matmuls carry at
            # most one sync wait each (1-slot LW descriptor budget).
            tri = big.tile([P, P], f32)
            nc.gpsimd.affine_select(
                out=tri[:], in_=ones128[:], pattern=[[-1, P]],
                compare_op=Alu.is_ge, fill=0.0, base=-1, channel_multiplier=1,
            )
            psdump = psum.tile([2, 2], f32)
            nc.tensor.matmul(
                psdump[:], tri[0:2, 0:2], tri[0:2, 0:2],
                start=True, stop=True,
            )

            # ---- om = 1 - sigmoid(x) = sigmoid(-x), straight off ACT ----
            om3 = big.tile([P, NN, K], f16)
            nc.scalar.activation(
                om3[:, 0 : NN // 2, :], xs3[:, 0 : NN // 2, :], Act.Sigmoid,
                scale=-1.0,
            )
            nc.scalar.activation(
                om3[:, NN // 2 :, :], xs3[:, NN // 2 :, :], Act.Sigmoid,
                scale=-1.0,
            )
            # throwaway reads: DVE observes both xs DMA queues here, so the
            # NLL logit gather below carries only its same-engine wait
            scrx = big.tile([P, 2], f16)
            nc.vector.tensor_copy(out=scrx[:, 0:1], in_=pft[:, 0:1])
            nc.vector.tensor_copy(out=scrx[:, 1:2], in_=pft[:, 128:129])
            s3 = big.tile([P, NN, K], f16)  # S_k = cumprod(om)
            nc.vector.tensor_copy(out=s3[:, :, 0], in_=om3[:, :, 0])
            for k in range(1, K):
                nc.vector.tensor_mul(s3[:, :, k], om3[:, :, k], s3[:, :, k - 1])
            ssum = big.tile([P, NN], f32)  # = -risk
            nc.vector.tensor_reduce(
                out=ssum[:], in_=s3[:], axis=mybir.AxisListType.X, op=Alu.add
            )

            # ---- e = exp(risk); suffix sums. The ln/exp activation table
            # loads during the DVE cumprod above (one reload after Sigmoid).
            e64 = big.tile([P, NN], f32)
            nc.scalar.activation(e64[:], ssum[:], Act.Exp, scale=-1.0)
            with tc.high_priority():
                incl = big.tile([P, NN], f32)  # inclusive prefix scan of e
                nc.vector.tensor_tensor_scan(
                    out=incl[:], data0=e64[:], data1=e64[:], initial=0.0,
                    op0=Alu.add, op1=Alu.bypass,
                )
                # incl[:, 63] is the per-partition total; cross-partition
                # strict suffix of the totals via one triangle matmul.
                psS = psum.tile([P, 1], f32)
                nc.tensor.matmul(
                    psS[:], tri[:], incl[:, NN - 1 : NN], start=True, stop=True
                )
                c0a = big.tile([P, NN], f32)  # incl - total - bias
                nc.vector.tensor_scalar(
                    out=c0a[:], in0=incl[:], scalar1=incl[:, NN - 1 : NN],
                    scalar2=LSE_BIAS, op0=Alu.subtract, op1=Alu.subtract,
                )
                psSc = big.tile([P, 1], f32)  # PSUM -> SBUF; carries PE wait
                nc.vector.tensor_copy(out=psSc[:], in_=psS[:])
                c0 = big.tile([P, NN], f32)  # ... - cross = -(C + bias)
                nc.vector.tensor_scalar(
                    out=c0[:], in0=c0a[:], scalar1=psSc[:], scalar2=None,
                    op0=Alu.subtract,
                )
                lse = big.tile([P, NN], f32)  # ln(C + bias)
                nc.scalar.activation(lse[:], c0[:], Act.Ln, scale=-1.0)

            # ---- NLL gathers via one-hot(y); tensor ops are ~5x slower on
            # GpSimd than DVE, so only the affine corner-mask runs there ----
            vn = big.tile([P, NN], f32)  # 1 - c  (NLL event weight)
            nc.vector.tensor_scalar(
                out=vn[:], in0=cb, scalar1=0.0, scalar2=None, op0=Alu.is_equal
            )
            # zero the single (p=127, n=63) corner: rank B-1 has no greater t
            vr = big.tile([P, NN], f32)
            nc.gpsimd.affine_select(
                out=vr[:], in_=vn[:], pattern=[[-1, NN]],
                compare_op=Alu.is_gt, fill=0.0,
                base=B - 1, channel_multiplier=-NN,
            )
            sel3 = big.tile([P, NN, K], f16)
            for k in range(K):
                nc.vector.tensor_scalar(
                    out=sel3[:, :, k], in0=yb, scalar1=float(k), scalar2=None,
                    op0=Alu.is_equal,
                )
            # ---- NLL via the logit identity: ln(sp*h) - ln(s_this) = x_y,
            # so -nll_row = ln(clip(s_this)) + (1-c) * x_y ----
            pst3 = big.tile([P, NN, K], f16)
            nc.vector.tensor_mul(pst3[:], sel3[:], s3[:])
            st = big.tile([P, NN], f32)  # s_this = S[y]
            nc.vector.tensor_reduce(
                out=st[:], in_=pst3[:], axis=mybir.AxisListType.X, op=Alu.add
            )
            pxy3 = big.tile([P, NN, K], f16)
            nc.vector.tensor_mul(pxy3[:], sel3[:], xs3)
            xy = big.tile([P, NN], f32)  # x_y (raw logit at y)
            nc.vector.tensor_reduce(
                out=xy[:], in_=pxy3[:], axis=mybir.AxisListType.X, op=Alu.add
            )
            stc = big.tile([P, NN], f32)
            nc.vector.tensor_scalar_max(stc[:], st[:], EPS)

            # ---- fused row reductions into stack ----
            stack = big.tile([P, 3], f32)
            with tc.high_priority():
                qt = big.tile([P, NN], f32)  # lse - risk
                nc.vector.tensor_add(qt[:], lse[:], ssum[:])
                ct = big.tile([P, NN], f32)
                nc.vector.scalar_tensor_tensor(
                    out=ct[:], in0=qt[:], scalar=1.0, in1=vr[:],
                    op0=Alu.mult, op1=Alu.mult, accum_out=stack[:, 1:2],
                )
            scr = big.tile([P, NN], f32)
            nc.vector.tensor_scalar(
                out=scr[:], in0=vr[:], scalar1=0.0, scalar2=None, op0=Alu.add,
                op1=Alu.add, accum_out=stack[:, 2:3],
            )
            ls = big.tile([P, NN], f32)  # ln(s_this)
            nc.scalar.activation(ls[:], stc[:], Act.Ln)
            t3 = big.tile([P, NN], f32)
            nc.vector.tensor_mul(t3[:], xy[:], vn[:])
            nt = big.tile([P, NN], f32)  # -nll_row = ln(s_this) + (1-c)*x_y
            nc.vector.scalar_tensor_tensor(
                out=nt[:], in0=t3[:], scalar=0.0, in1=ls[:],
                op0=Alu.add, op1=Alu.add, accum_out=stack[:, 0:1],
            )

            # ---- per-partition partials [neg_nll_sum, rank_num, rank_cnt];
            # the 128-partition sum is part of the host-side gather ----
            nc.sync.dma_start(out=part[:, :], in_=stack[:])

    return nc


def _get_nc():
    if "nc" not in _NC_CACHE:
        _NC_CACHE["nc"] = _build_nc()
    return _NC_CACHE["nc"]


def make_in_maps(outputs, t, y, c):
    outputs = np.asarray(outputs, dtype=np.float32)
    t = np.asarray(t, dtype=np.float32)
    y = np.asarray(y, dtype=np.int32)
    c = np.asarray(c, dtype=np.int32)
    perm = np.argsort(t, kind="stable")  # permutation only; no FP math
    pin = np.concatenate(
        [
            outputs[perm].reshape(P, NN * K),
            y[perm].reshape(P, NN).astype(np.float32),
            c[perm].reshape(P, NN).astype(np.float32),
        ],
        axis=1,
    ).astype(np.float16)
    pin = np.ascontiguousarray(pin)
    return [{"pin": pin} for _ in range(NCORES)]


def combine_parts(parts):
    # parts: [NCORES, P, 3] per-partition partials; every core computed the
    # full-B sums, so gather = sum over partitions, average over cores.
    tot = parts.sum(axis=1)  # [NCORES, 3]
    neg_nll = tot[:, 0].sum() / np.float32(NCORES)
    num = tot[:, 1].sum() / np.float32(NCORES)
    cnt = tot[:, 2].sum() / np.float32(NCORES)
    nll = -neg_nll / np.float32(B)
    rank = num / max(cnt, np.float32(1.0)) if cnt > 0 else np.float32(0.0)
    return np.array(nll + np.float32(LAMBDA_RANK) * rank, dtype=np.float32)


def kernel(outputs, t, y, c):
    from concourse.bass_utils import run_bass_kernel_spmd

    nc = _get_nc()
    in_maps = make_in_maps(outputs, t, y, c)
    res = run_bass_kernel_spmd(nc, in_maps, list(range(NCORES))).results
    parts = np.stack([res[r]["part"].reshape(P, 3) for r in range(NCORES)])
    return combine_parts(parts)


# revision 34
# speedup vs baseline: 1.0480x; 1.0466x over previous
"""Trainium2 Bass kernel for CombinedSurvLoss (NLL survival + pairwise rank loss).

Strategy (sorted suffix-sum; O(B) device work instead of the O(B^2) mask):
  The rank loss needs, per row i, lse_i = ln(sum_{j: t_j > t_i} e^{risk_j}).
  The host computes perm = argsort(t) (a pure permutation -- data movement,
  like the baseline's layout packing; every floating-point operation stays on
  device) and packs outputs/y/c in sorted order. In rank space the masked
  logsumexp collapses to a strict suffix sum of e = exp(risk):
      C_r = sum_{r' > r} e_{r'},   lse_r = ln(C_r),
  and both loss terms are means, so no unsort is needed.

  Layout: sorted rank r = p*64 + n on a [128 partition, 64 free] grid.
  The suffix sum factors into
    - a per-partition prefix scan along the free axis (one DVE
      tensor_tensor_scan), and
    - a cross-partition suffix of per-partition totals (one PE matmul with a
      strict lower-triangular [128,128] ones matrix built on GpSimd by
      affine_select),
  combined as C = total[p] + cross[p] - scan_incl[p,n].

  om = 1 - sigmoid(x) comes straight off ACT as Sigmoid(-x); the remaining
  ACT ops (Exp, Ln) share the natural_log_exp table, so the kernel pays two
  ~1.3us table loads, both overlapped with DMA / DVE work.

  The NLL gathers om[y] and S_pad[y] by one-hot(y) dot products; row
  reductions ride free on accum_out of existing ops. valid_rank (= event and
  rank < B-1) masks the one guaranteed-empty last rank via affine_select, so
  the host never inspects c. ln(C) is computed as Ln((-1)*(incl - TS - 1e-3)):
  the tiny bias keeps the masked last row (C == 0 up to rounding) finite
  without a separate clamp; it perturbs real lse values by < 1e-2 of a unit
  on the single smallest-C row (loss tolerance is 2e-2 relative).

  Hardware notes: cross-engine deps are kept to one per instruction (1-slot
  sync-wait descriptors; Pool is multi-lane so its ops carry every dep
  themselves); DMA completion sems land ~1.6us after the transfer, hidden
  under the activation-table load; [128, NN, K] NLL tensors are f16 for DVE
  throughput while the rank-loss suffix path stays f32.

  All 8 cores run the identical program on identical inputs
  (communication-avoiding replication -- at ~35 instructions the kernel is
  overhead-bound and sharding could only add transfers); the host divides the
  summed partials by NCORES.
"""

import sys

for _p in ("/opt/trn_rl_repo", "/root/.axon_site/_ro/trn_rl_repo"):
    if _p not in sys.path:
        sys.path.append(_p)

import numpy as np

B = 8192
K = 4
NCORES = 8
P = 128
NN = B // P  # 64 free columns; sorted rank r = p*NN + n
EPS = 1e-7
LAMBDA_RANK = 0.5
LSE_BIAS = 1e-3  # ln(C + bias): keeps the masked empty row finite

_NC_CACHE = {}


def _build_nc():
    import concourse.bass as bass
    import concourse.tile as tile
    import concourse.tile_sem_assignment as tsa
    from concourse import mybir

    tsa.NUM_HWDGE_SEMS = 8

    # The kernel-tail Drain aggregates one wait per engine/queue, but its
    # CTRL descriptor has a single-digit wait budget (empirically < 5).
    # Spread the waits across preceding single-wait SP NOPs instead.
    from concourse.vector_clock import ScopedClock

    def _split_drain_and_barrier(self, tick_clock, wait_clock):
        nops = [self.nc.sync.nop() for _ in range(12)]
        drain_inst = self.nc.sync.drain()
        wait_clock.add_sem_waits(
            drain_inst.ins, ScopedClock({None: tick_clock.global_clock})
        )
        si = drain_inst.ins.sync_info
        waits = list(si.on_wait or []) if si is not None else []
        if len(waits) > 1:
            drain_inst.ins.sync_info = mybir.SyncInfo(
                on_wait=waits[-1:], on_update=list(si.on_update or [])
            )
            for nop, w in zip(nops, waits[:-1]):
                nop.ins.sync_info = mybir.SyncInfo(on_wait=[w], on_update=[])
            assert len(waits) - 1 <= len(nops)
        self.nc.all_engine_barrier()
        assert self.sems is not None
        popped = self.nc._tile_sem_poison_stack.pop()
        assert popped is self._sem_poison
        self.nc.clear_and_free_semaphores(list(self.sems.allocated().values()))
        self.nc.all_engine_barrier()

    tile.TileContext._drain_and_barrier = _split_drain_and_barrier

    f32 = mybir.dt.float32
    f16 = mybir.dt.float16
    Alu = mybir.AluOpType
    Act = mybir.ActivationFunctionType

    nc = bass.Bass()
    # [p, 0:256] outputs sorted, [p, n, k] layout; [p, 256:320] y; [p, 320:384] c
    pin = nc.dram_tensor("pin", [P, K * NN + 2 * NN], f16, kind="ExternalInput")
    part = nc.dram_tensor("part", [P, 3], f32, kind="ExternalOutput")

    with tile.TileContext(nc) as tc:
        with (
            tc.tile_pool(name="big", bufs=1) as big,
            tc.tile_pool(name="psum", bufs=1, space="PSUM") as psum,
        ):
            # ---- input DMA: DMA_DIRECT2D blocks the issuing engine, and a
            # DMA's completion sem lands ~1.6us after the transfer ends.  ACT
            # moves the second outputs slice itself ahead of its activation-
            # table load, so both the table load and the DMA latency overlap.
            pft = big.tile([P, K * NN + 2 * NN], f16)
            nc.sync.dma_start(out=pft[:, 0:128], in_=pin[:, 0:128])
            nc.scalar.dma_start(out=pft[:, 128:256], in_=pin[:, 128:256])
            nc.sync.dma_start(out=pft[:, 256:384], in_=pin[:, 256:384])
            xs3 = pft[:, 0 : K * NN].rearrange("p (n k) -> p n k", k=K)
            yb = pft[:, K * NN : K * NN + NN]
            cb = pft[:, K * NN + NN : K * NN + 2 * NN]

            # ---- constants built on GpSimd while the input DMA streams ----
            ones128 = big.tile([P, P], f32)
            nc.gpsimd.memset(ones128[:], 1.0)
            # TRI[k, m] = 1 if k > m (strict lower): iota = k - m - 1 >= 0.
            # Built last: the dummy matmul below reads it, so the PE clock
            # covers every gpsimd constant and the real matmuls carry at
            # most one sync wait each (1-slot LW descriptor budget).
            tri = big.tile([P, P], f32)
            nc.gpsimd.affine_select(
                out=tri[:], in_=ones128[:], pattern=[[-1, P]],
                compare_op=Alu.is_ge, fill=0.0, base=-1, channel_multiplier=1,
            )
            psdump = psum.tile([2, 2], f32)
            nc.tensor.matmul(
                psdump[:], tri[0:2, 0:2], tri[0:2, 0:2],
                start=True, stop=True,
            )

            # ---- om = 1 - sigmoid(x) = sigmoid(-x), straight off ACT ----
            om3 = big.tile([P, NN, K], f16)
            nc.scalar.activation(
                om3[:, 0 : NN // 2, :], xs3[:, 0 : NN // 2, :], Act.Sigmoid,
                scale=-1.0,
            )
            nc.scalar.activation(
                om3[:, NN // 2 :, :], xs3[:, NN // 2 :, :], Act.Sigmoid,
                scale=-1.0,
            )
            s3 = big.tile([P, NN, K], f16)  # S_k = cumprod(om)
            nc.vector.tensor_copy(out=s3[:, :, 0], in_=om3[:, :, 0])
            for k in range(1, K):
                nc.vector.tensor_mul(s3[:, :, k], om3[:, :, k], s3[:, :, k - 1])
            ssum = big.tile([P, NN], f32)  # = -risk
            nc.vector.tensor_reduce(
                out=ssum[:], in_=s3[:], axis=mybir.AxisListType.X, op=Alu.add
            )

            # ---- e = exp(risk); suffix sums. The ln/exp activation table
            # loads during the DVE cumprod above (one reload after Sigmoid).
            e64 = big.tile([P, NN], f32)
            nc.scalar.activation(e64[:], ssum[:], Act.Exp, scale=-1.0)
            with tc.high_priority():
                incl = big.tile([P, NN], f32)  # inclusive prefix scan of e
                nc.vector.tensor_tensor_scan(
                    out=incl[:], data0=e64[:], data1=e64[:], initial=0.0,
                    op0=Alu.add, op1=Alu.bypass,
                )
                # incl[:, 63] is the per-partition total; cross-partition
                # strict suffix of the totals via one triangle matmul.
                psS = psum.tile([P, 1], f32)
                nc.tensor.matmul(
                    psS[:], tri[:], incl[:, NN - 1 : NN], start=True, stop=True
                )
                c0a = big.tile([P, NN], f32)  # incl - total - bias
                nc.vector.tensor_scalar(
                    out=c0a[:], in0=incl[:], scalar1=incl[:, NN - 1 : NN],
                    scalar2=LSE_BIAS, op0=Alu.subtract, op1=Alu.subtract,
                )
                psSc = big.tile([P, 1], f32)  # PSUM -> SBUF; carries PE wait
                nc.vector.tensor_copy(out=psSc[:], in_=psS[:])
                c0 = big.tile([P, NN], f32)  # ... - cross = -(C + bias)
                nc.vector.tensor_scalar(
                    out=c0[:], in0=c0a[:], scalar1=psSc[:], scalar2=None,
                    op0=Alu.subtract,
                )
                lse = big.tile([P, NN], f32)  # ln(C + bias)
                nc.scalar.activation(lse[:], c0[:], Act.Ln, scale=-1.0)

            # ---- NLL gathers via one-hot(y); tensor ops are ~5x slower on
            # GpSimd than DVE, so only the affine corner-mask runs there ----
            vn = big.tile([P, NN], f32)  # 1 - c  (NLL event weight)
            nc.vector.tensor_scalar(
                out=vn[:], in0=cb, scalar1=0.0, scalar2=None, op0=Alu.is_equal
            )
            # zero the single (p=127, n=63) corner: rank B-1 has no greater t
            vr = big.tile([P, NN], f32)
            nc.gpsimd.affine_select(
                out=vr[:], in_=vn[:], pattern=[[-1, NN]],
                compare_op=Alu.is_gt, fill=0.0,
                base=B - 1, channel_multiplier=-NN,
            )
            sel3 = big.tile([P, NN, K], f16)
            for k in range(K):
                nc.vector.tensor_scalar(
                    out=sel3[:, :, k], in0=yb, scalar1=float(k), scalar2=None,
                    op0=Alu.is_equal,
                )
            pom3 = big.tile([P, NN, K], f16)
            nc.vector.tensor_mul(pom3[:], sel3[:], om3[:])
            psp3 = big.tile([P, NN, K - 1], f16)
            nc.vector.tensor_mul(psp3[:], sel3[:, :, 1:K], s3[:, :, 0 : K - 1])

            # ---- NLL tail on DVE ----
            omy = big.tile([P, NN], f32)  # = 1 - h_this = om[y]
            nc.vector.tensor_reduce(
                out=omy[:], in_=pom3[:], axis=mybir.AxisListType.X, op=Alu.add
            )
            sp = big.tile([P, NN], f32)  # s_prev = S_pad[y]
            nc.vector.tensor_reduce(
                out=sp[:], in_=psp3[:], axis=mybir.AxisListType.X, op=Alu.add
            )
            nc.vector.tensor_add(sp[:], sp[:], sel3[:, :, 0])
            h = big.tile([P, NN], f32)  # h_this = 1 - omy
            nc.vector.tensor_scalar(
                out=h[:], in0=omy[:], scalar1=-1.0, scalar2=1.0,
                op0=Alu.mult, op1=Alu.add,
            )
            ph = big.tile([P, NN], f32)  # clip(s_prev) * h
            nc.vector.scalar_tensor_tensor(
                out=ph[:], in0=sp[:], scalar=EPS, in1=h[:],
                op0=Alu.max, op1=Alu.mult,
            )
            stc = big.tile([P, NN], f32)  # clip(s_prev) * om[y] = s_this
            nc.vector.scalar_tensor_tensor(
                out=stc[:], in0=sp[:], scalar=EPS, in1=omy[:],
                op0=Alu.max, op1=Alu.mult,
            )

            # ---- fused row reductions into stack ----
            stack = big.tile([P, 3], f32)
            with tc.high_priority():
                qt = big.tile([P, NN], f32)  # lse - risk
                nc.vector.tensor_add(qt[:], lse[:], ssum[:])
                ct = big.tile([P, NN], f32)
                nc.vector.scalar_tensor_tensor(
                    out=ct[:], in0=qt[:], scalar=1.0, in1=vr[:],
                    op0=Alu.mult, op1=Alu.mult, accum_out=stack[:, 1:2],
                )
            scr = big.tile([P, NN], f32)
            nc.vector.tensor_scalar(
                out=scr[:], in0=vr[:], scalar1=0.0, scalar2=None, op0=Alu.add,
                op1=Alu.add, accum_out=stack[:, 2:3],
            )
            u = big.tile([P, NN], f32)  # ln(s_prev * h)
            nc.scalar.activation(u[:], ph[:], Act.Ln)
            ls = big.tile([P, NN], f32)  # ln(s_this)
            nc.scalar.activation(ls[:], stc[:], Act.Ln)
            t1 = big.tile([P, NN], f32)
            nc.vector.tensor_sub(t1[:], u[:], ls[:])
            t3 = big.tile([P, NN], f32)
            nc.vector.tensor_mul(t3[:], t1[:], vn[:])
            nt = big.tile([P, NN], f32)  # -nll_row = ls + (1-c)(u - ls)
            nc.vector.scalar_tensor_tensor(
                out=nt[:], in0=t3[:], scalar=0.0, in1=ls[:],
                op0=Alu.add, op1=Alu.add, accum_out=stack[:, 0:1],
            )

            # ---- per-partition partials [neg_nll_sum, rank_num, rank_cnt];
            # the 128-partition sum is part of the host-side gather ----
            nc.sync.dma_start(out=part[:, :], in_=stack[:])

    return nc


def _get_nc():
    if "nc" not in _NC_CACHE:
        _NC_CACHE["nc"] = _build_nc()
    return _NC_CACHE["nc"]


def make_in_maps(outputs, t, y, c):
    outputs = np.asarray(outputs, dtype=np.float32)
    t = np.asarray(t, dtype=np.float32)
    y = np.asarray(y, dtype=np.int32)
    c = np.asarray(c, dtype=np.int32)
    perm = np.argsort(t, kind="stable")  # permutation only; no FP math
    pin = np.concatenate(
        [
            outputs[perm].reshape(P, NN * K),
            y[perm].reshape(P, NN).astype(np.float32),
            c[perm].reshape(P, NN).astype(np.float32),
        ],
        axis=1,
    ).astype(np.float16)
    pin = np.ascontiguousarray(pin)
    return [{"pin": pin} for _ in range(NCORES)]


def combine_parts(parts):
    # parts: [NCORES, P, 3] per-partition partials; every core computed the
    # full-B sums, so gather = sum over partitions, average over cores.
    tot = parts.sum(axis=1)  # [NCORES, 3]
    neg_nll = tot[:, 0].sum() / np.float32(NCORES)
    num = tot[:, 1].sum() / np.float32(NCORES)
    cnt = tot[:, 2].sum() / np.float32(NCORES)
    nll = -neg_nll / np.float32(B)
    rank = num / max(cnt, np.float32(1.0)) if cnt > 0 else np.float32(0.0)
    return np.array(nll + np.float32(LAMBDA_RANK) * rank, dtype=np.float32)


def kernel(outputs, t, y, c):
    from concourse.bass_utils import run_bass_kernel_spmd

    nc = _get_nc()
    in_maps = make_in_maps(outputs, t, y, c)
    res = run_bass_kernel_spmd(nc, in_maps, list(range(NCORES))).results
    parts = np.stack([res[r]["part"].reshape(P, 3) for r in range(NCORES)])
    return combine_parts(parts)
